# revision 7
# baseline (speedup 1.0000x reference)
"""Trainium2 Bass kernel for nn_BOW_model (fused EmbeddingBag-mean -> Linear
-> BatchNorm1d -> ReLU -> Linear -> BCEWithLogitsLoss).

Sharding: data-parallel over documents. Each of the 8 cores owns 512
contiguous documents (segment ids are sorted). The embedding table and MLP
params are replicated. Per core: gather its tokens' embedding rows from HBM
(Ant dma_gather, int16 indices, so tokens are bucketed into four 32768-row
vocab windows and sorted within each window), segment-sum them into per-doc
sums + counts with a one-hot matmul on the tensor engine, then the tiny MLP.
BN batch stats (sum h, sum h^2) and the BCE loss sum are all-reduced across
cores on-device. Host only shards inputs / concatenates outputs.
"""
import numpy as np

import concourse.bacc as bacc
import concourse.mybir as mybir
import concourse.tile as tile
from concourse.masks import make_identity

VOCAB = 100000
HID = 256
BATCH = 4096
TOTAL_TOKENS = 819200
BN_EPS = 1e-5
N_CORES = 8
DOCS_PER_CORE = BATCH // N_CORES  # 512
CHUNK_DOCS = 128  # docs per PSUM accumulation chunk
N_CHUNKS = BATCH // CHUNK_DOCS  # 32 global, 4 per core
WIN = 32768  # vocab rows reachable by one int16 index window
N_WIN = 4  # ceil(100000 / 32768)
SUB = 8  # 128-row tiles per dma_gather call


# ----------------------------------------------------------------------------
# Workaround: this walrus build allows a single sync-wait command on the
# kernel-tail Drain, but Tile attaches one wait per in-flight proc. Absorb
# each pending wait into its own SP NOP ahead of a wait-free drain.
# ----------------------------------------------------------------------------
import bass_rust as _br


class _TileContextFixed(tile.TileContext):
    def _drain_and_barrier(self, tick_clock, wait_clock):
        gc = tick_clock.global_clock
        n = _br.N_PROCS
        for p in range(n):
            if gc[p] <= 0:
                continue
            partial = _br.VectorClock([gc[i] if i == p else 0 for i in range(n)])
            nop = self.nc.sync.nop(nofuse=True, hint=f"drain_absorb_p{p}")
            wait_clock.add_sem_waits(nop.ins, _br.ScopedClock({None: partial}))
        self.nc.sync.drain()
        self.nc.all_engine_barrier()
        assert self.sems is not None
        popped = self.nc._tile_sem_poison_stack.pop()
        assert popped is self._sem_poison
        self.nc.clear_and_free_semaphores(list(self.sems.allocated().values()))
        self.nc.all_engine_barrier()


# ----------------------------------------------------------------------------
# Device program
# ----------------------------------------------------------------------------
def build_bass(lw: tuple[int, ...], debug: bool = False):
    """lw[w] = padded token count per (chunk, vocab-window) group (mult of 128).

    Per-core inputs:
      emb    [VOCAB, HID] f32      (replicated)
      idx16  [128, 4*sum(lw)//16] int16  window-relative token ids, wrapped
             i -> [i%16, i//16] per group, groups ordered (chunk, w)
      segf   [128, 4*sum(lw)//128] f32   chunk-relative segment ids, wrapped
             i -> [i%128, i//128] per group (pad = -1)
      w1t    [HID, HID] f32  (= W1.T)
      b1c/gammac/betac/w2c [HID, 1] f32
      b2c    [1, 1] f32
      labels [1, DOCS_PER_CORE] f32
      iota   [128, 128] f32, iota[p, m] = m
    Outputs:
      logits [1, DOCS_PER_CORE] f32
      loss   [1, 1] f32 (already divided by BATCH; identical on all cores)
    """
    f32 = mybir.dt.float32
    group_tiles = [l // 128 for l in lw]
    tiles_per_chunk = sum(group_tiles)
    n_idx_cols = 4 * sum(lw) // 16
    n_seg_cols = 4 * tiles_per_chunk

    nc = bacc.Bacc("TRN2", target_bir_lowering=False, num_swdge_queues=4)
    emb = nc.declare_dram_parameter("emb", [VOCAB, HID], f32, isOutput=False)
    idx16 = nc.declare_dram_parameter(
        "idx16", [128, n_idx_cols], mybir.dt.int16, isOutput=False
    )
    segf = nc.declare_dram_parameter("segf", [128, n_seg_cols], f32, isOutput=False)
    w1t = nc.declare_dram_parameter("w1t", [HID, HID], f32, isOutput=False)
    b1c = nc.declare_dram_parameter("b1c", [HID, 1], f32, isOutput=False)
    gammac = nc.declare_dram_parameter("gammac", [HID, 1], f32, isOutput=False)
    betac = nc.declare_dram_parameter("betac", [HID, 1], f32, isOutput=False)
    w2c = nc.declare_dram_parameter("w2c", [HID, 1], f32, isOutput=False)
    b2c = nc.declare_dram_parameter("b2c", [1, 1], f32, isOutput=False)
    labels = nc.declare_dram_parameter(
        "labels", [1, DOCS_PER_CORE], f32, isOutput=False
    )
    iota_in = nc.declare_dram_parameter("iota", [128, 128], f32, isOutput=False)
    out_logits = nc.declare_dram_parameter(
        "logits", [1, DOCS_PER_CORE], f32, isOutput=True
    )
    out_loss = nc.declare_dram_parameter("loss", [1, 1], f32, isOutput=True)
    if debug:
        out_bow = nc.declare_dram_parameter("dbg_bow", [128, 4 * HID + 4], f32, isOutput=True)
        out_h = nc.declare_dram_parameter("dbg_h", [HID, DOCS_PER_CORE], f32, isOutput=True)

    AF = mybir.ActivationFunctionType
    ALU = mybir.AluOpType
    AX = mybir.AxisListType

    with _TileContextFixed(nc) as tc:
        with (
            tc.tile_pool(name="const", bufs=1) as cp,
            tc.tile_pool(name="gat", bufs=6) as gp,
            tc.tile_pool(name="oh", bufs=4) as ohp,
            tc.tile_pool(name="small", bufs=2) as sp,
            tc.tile_pool(name="psA", bufs=1, space="PSUM") as psA,
            tc.tile_pool(name="psT", bufs=2, space="PSUM") as psT,
            tc.tile_pool(name="psH", bufs=1, space="PSUM") as psH,
            tc.tile_pool(name="psL", bufs=1, space="PSUM") as psL,
            tc.tile_pool(name="dram", bufs=1, space="DRAM") as dp,
        ):
            # ---- constants / static tiles ----
            iota_sb = cp.tile([128, 128], f32, name="iota")
            nc.sync.dma_start(out=iota_sb[:], in_=iota_in[:])
            ones_sb = cp.tile([128, 1], f32, name="ones")
            nc.vector.memset(ones_sb[:], 1.0)
            ident = cp.tile([128, 128], f32, name="ident")
            make_identity(nc, ident[:])
            idx_sb = cp.tile([128, n_idx_cols], mybir.dt.int16, name="idx")
            nc.sync.dma_start(out=idx_sb[:], in_=idx16[:])
            seg_sb = cp.tile([128, n_seg_cols], f32, name="seg")
            nc.sync.dma_start(out=seg_sb[:], in_=segf[:])
            w1t_sb = [cp.tile([128, HID], f32, name=f"w1t{ih}") for ih in range(2)]
            for ih in range(2):
                nc.sync.dma_start(
                    out=w1t_sb[ih][:], in_=w1t[ih * 128 : (ih + 1) * 128, :]
                )
            b1_sb = [cp.tile([128, 1], f32, name=f"b1{j}") for j in range(2)]
            gm_sb = [cp.tile([128, 1], f32, name=f"gm{j}") for j in range(2)]
            bt_sb = [cp.tile([128, 1], f32, name=f"bt{j}") for j in range(2)]
            w2_sb = [cp.tile([128, 1], f32, name=f"w2{j}") for j in range(2)]
            for j in range(2):
                rows = slice(j * 128, (j + 1) * 128)
                nc.sync.dma_start(out=b1_sb[j][:], in_=b1c[rows, :])
                nc.sync.dma_start(out=gm_sb[j][:], in_=gammac[rows, :])
                nc.sync.dma_start(out=bt_sb[j][:], in_=betac[rows, :])
                nc.sync.dma_start(out=w2_sb[j][:], in_=w2c[rows, :])
            b2_sb = cp.tile([1, 1], f32, name="b2")
            nc.sync.dma_start(out=b2_sb[:], in_=b2c[:])
            lab_sb = cp.tile([1, DOCS_PER_CORE], f32, name="lab")
            nc.sync.dma_start(out=lab_sb[:], in_=labels[:])

            bow_sb = cp.tile([128, 4 * HID], f32, name="bow")
            eps_sb = cp.tile([128, 1], f32, name="eps")
            nc.vector.memset(eps_sb[:], BN_EPS)

            # ---- phase 1: gather + segment-sum per 128-doc chunk ----
            qn = 0
            for ch in range(4):
                ps = psA.tile([128, HID + 1], f32, name="acc")
                ohsum = sp.tile([128, 128], f32, name="ohsum")
                nc.vector.memset(ohsum[:], 0.0)
                tile_col0 = ch * tiles_per_chunk  # seg col base for this chunk
                idx_col0 = ch * sum(lw) // 16
                n_tiles_done = 0
                for w in range(4):
                    l = lw[w]
                    g_tiles = group_tiles[w]
                    # gather calls of up to SUB tiles each
                    emb_win = emb[w * WIN :, :]
                    done = 0
                    while done < g_tiles:
                        ct = min(SUB, g_tiles - done)
                        ni = ct * 128
                        g = gp.tile([128, SUB * HID], f32, name="g")
                        nc.gpsimd.dma_gather(
                            out_ap=g[:, : ct * HID].rearrange(
                                "p (s h) -> p s h", s=ct
                            ),
                            in_ap=emb_win,
                            idxs_ap=idx_sb[
                                :, idx_col0 + done * 8 : idx_col0 + (done + ct) * 8
                            ],
                            num_idxs=ni,
                            num_idxs_reg=ni,
                            elem_size=HID,
                            queue_num=qn,
                            single_packet=False,
                        )
                        qn = (qn + 1) % 4
                        for s in range(ct):
                            tcol = tile_col0 + n_tiles_done
                            oh = ohp.tile([128, 128], f32, name="oh")
                            nc.vector.tensor_scalar(
                                out=oh[:],
                                in0=iota_sb[:],
                                scalar1=seg_sb[:, tcol : tcol + 1],
                                scalar2=None,
                                op0=ALU.is_equal,
                            )
                            first = n_tiles_done == 0
                            last = n_tiles_done == tiles_per_chunk - 1
                            nc.tensor.matmul(
                                out=ps[:, :HID],
                                lhsT=oh[:],
                                rhs=g[:, s * HID : (s + 1) * HID],
                                start=first,
                                stop=last,
                            )
                            nc.vector.tensor_add(
                                out=ohsum[:], in0=ohsum[:], in1=oh[:]
                            )
                            n_tiles_done += 1
                        done += ct
                    idx_col0 += l // 16
                nc.tensor.matmul(
                    out=ps[:, HID : HID + 1],
                    lhsT=ohsum[:],
                    rhs=ones_sb[:],
                    start=True,
                    stop=True,
                )
                # bow = seg_sum / max(counts, 1)
                cnt = sp.tile([128, 1], f32, name="cnt")
                nc.vector.tensor_scalar_max(
                    out=cnt[:], in0=ps[:, HID : HID + 1], scalar1=1.0
                )
                if debug:
                    nc.sync.dma_start(
                        out=out_bow[:, 4 * HID + ch : 4 * HID + ch + 1], in_=cnt[:]
                    )
                rec = sp.tile([128, 1], f32, name="rec")
                nc.vector.reciprocal(out=rec[:], in_=cnt[:])
                nc.vector.tensor_scalar_mul(
                    out=bow_sb[:, ch * HID : (ch + 1) * HID],
                    in0=ps[:, :HID],
                    scalar1=rec[:],
                )

            if debug:
                nc.sync.dma_start(out=out_bow[:, : 4 * HID], in_=bow_sb[:])

            # ---- transpose bow [512 docs, 256] -> bowT (2 x [128, 512]) ----
            bowT = [cp.tile([128, 4 * 128], f32, name=f"bowT{j}") for j in range(2)]
            for ch in range(4):
                for fh in range(2):
                    pt = psT.tile([128, 128], f32, name="pt")
                    nc.tensor.transpose(
                        out=pt[:],
                        in_=bow_sb[:, ch * HID + fh * 128 : ch * HID + (fh + 1) * 128],
                        identity=ident[:],
                    )
                    nc.vector.tensor_copy(
                        out=bowT[fh][:, ch * 128 : (ch + 1) * 128], in_=pt[:]
                    )

            # ---- h^T = W1T @ bowT + b1; local BN stats ----
            h_sb = [cp.tile([128, DOCS_PER_CORE], f32, name=f"h{j}") for j in range(2)]
            stats = cp.tile([128, 4], f32, name="stats")
            sq = sp.tile([128, DOCS_PER_CORE], f32, name="sq")
            for jh in range(2):
                ph = psH.tile([128, DOCS_PER_CORE], f32, name="ph")
                for ih in range(2):
                    nc.tensor.matmul(
                        out=ph[:],
                        lhsT=w1t_sb[ih][:, jh * 128 : (jh + 1) * 128],
                        rhs=bowT[ih][:],
                        start=ih == 0,
                        stop=ih == 1,
                    )
                nc.vector.tensor_scalar_add(
                    out=h_sb[jh][:], in0=ph[:], scalar1=b1_sb[jh][:]
                )
                if debug:
                    nc.sync.dma_start(
                        out=out_h[jh * 128 : (jh + 1) * 128, :], in_=h_sb[jh][:]
                    )
                nc.vector.tensor_reduce(
                    out=stats[:, jh : jh + 1],
                    in_=h_sb[jh][:],
                    axis=AX.X,
                    op=ALU.add,
                )
                nc.vector.tensor_mul(out=sq[:], in0=h_sb[jh][:], in1=h_sb[jh][:])
                nc.vector.tensor_reduce(
                    out=stats[:, 2 + jh : 3 + jh], in_=sq[:], axis=AX.X, op=ALU.add
                )

            # ---- all-reduce BN stats across the 8 cores ----
            st_in = dp.tile([128, 4], f32, name="st_in")
            st_out = dp.tile([128, 4], f32, name="st_out")
            nc.sync.dma_start(out=st_in[:], in_=stats[:])
            nc.gpsimd.collective_compute(
                "AllReduce",
                ALU.add,
                replica_groups=[list(range(N_CORES))],
                ins=[st_in.opt()],
                outs=[st_out.opt()],
            )
            stg = cp.tile([128, 4], f32, name="stg")
            nc.sync.dma_start(out=stg[:], in_=st_out[:])

            # ---- BN scale/shift; hn = relu(h*scale + shift); logits ----
            pl = psL.tile([1, DOCS_PER_CORE], f32, name="pl")
            hn = sp.tile([128, DOCS_PER_CORE], f32, name="hn")
            for jh in range(2):
                mu = sp.tile([128, 1], f32, name="mu")
                nc.vector.tensor_scalar_mul(
                    out=mu[:], in0=stg[:, jh : jh + 1], scalar1=1.0 / BATCH
                )
                ex2 = sp.tile([128, 1], f32, name="ex2")
                nc.vector.tensor_scalar_mul(
                    out=ex2[:], in0=stg[:, 2 + jh : 3 + jh], scalar1=1.0 / BATCH
                )
                var = sp.tile([128, 1], f32, name="var")
                nc.vector.tensor_mul(out=var[:], in0=mu[:], in1=mu[:])
                nc.vector.tensor_sub(out=var[:], in0=ex2[:], in1=var[:])
                sd = sp.tile([128, 1], f32, name="sd")
                # sd = sqrt(var + eps)
                nc.scalar.activation(out=sd[:], in_=var[:], func=AF.Sqrt, bias=eps_sb[:])
                rsd = sp.tile([128, 1], f32, name="rsd")
                nc.vector.reciprocal(out=rsd[:], in_=sd[:])
                scale = sp.tile([128, 1], f32, name="scale")
                nc.vector.tensor_mul(out=scale[:], in0=gm_sb[jh][:], in1=rsd[:])
                shift = sp.tile([128, 1], f32, name="shift")
                nc.vector.tensor_mul(out=shift[:], in0=mu[:], in1=scale[:])
                nc.vector.tensor_sub(out=shift[:], in0=bt_sb[jh][:], in1=shift[:])
                nc.scalar.activation(
                    out=hn[:],
                    in_=h_sb[jh][:],
                    func=AF.Relu,
                    bias=shift[:],
                    scale=scale[:],
                )
                nc.tensor.matmul(
                    out=pl[:],
                    lhsT=w2_sb[jh][:],
                    rhs=hn[:],
                    start=jh == 0,
                    stop=jh == 1,
                )

            logit = sp.tile([1, DOCS_PER_CORE], f32, name="logit")
            nc.vector.tensor_scalar_add(out=logit[:], in0=pl[:], scalar1=b2_sb[:])
            nc.sync.dma_start(out=out_logits[:], in_=logit[:])

            # ---- BCE loss: max(l,0) - l*y + softplus(-|l|) ----
            t_relu = sp.tile([1, DOCS_PER_CORE], f32, name="t_relu")
            nc.scalar.activation(out=t_relu[:], in_=logit[:], func=AF.Relu)
            t_ly = sp.tile([1, DOCS_PER_CORE], f32, name="t_ly")
            nc.vector.tensor_mul(out=t_ly[:], in0=logit[:], in1=lab_sb[:])
            t_abs = sp.tile([1, DOCS_PER_CORE], f32, name="t_abs")
            nc.scalar.activation(out=t_abs[:], in_=logit[:], func=AF.Abs)
            t_e = sp.tile([1, DOCS_PER_CORE], f32, name="t_e")
            nc.scalar.activation(out=t_e[:], in_=t_abs[:], func=AF.Exp, scale=-1.0)
            t_sp = sp.tile([1, DOCS_PER_CORE], f32, name="t_sp")
            nc.scalar.activation(out=t_sp[:], in_=t_e[:], func=AF.Ln, bias=1.0)
            nc.vector.tensor_sub(out=t_relu[:], in0=t_relu[:], in1=t_ly[:])
            nc.vector.tensor_add(out=t_relu[:], in0=t_relu[:], in1=t_sp[:])
            lsum = sp.tile([1, 1], f32, name="lsum")
            nc.vector.tensor_reduce(
                out=lsum[:], in_=t_relu[:], axis=AX.X, op=ALU.add
            )
            ls_in = dp.tile([1, 1], f32, name="ls_in")
            ls_out = dp.tile([1, 1], f32, name="ls_out")
            nc.sync.dma_start(out=ls_in[:], in_=lsum[:])
            nc.gpsimd.collective_compute(
                "AllReduce",
                ALU.add,
                replica_groups=[list(range(N_CORES))],
                ins=[ls_in.opt()],
                outs=[ls_out.opt()],
            )
            lsg = sp.tile([1, 1], f32, name="lsg")
            nc.sync.dma_start(out=lsg[:], in_=ls_out[:])
            nc.vector.tensor_scalar_mul(
                out=lsg[:], in0=lsg[:], scalar1=1.0 / BATCH
            )
            nc.sync.dma_start(out=out_loss[:], in_=lsg[:])

    nc.compile()
    return nc


# ----------------------------------------------------------------------------
# PJRT runner (kept warm across kernel() calls)
# ----------------------------------------------------------------------------
class _Runner:
    def __init__(self, nc, n_cores):
        import jax
        from jax.sharding import Mesh, PartitionSpec
        from jax.experimental.shard_map import shard_map
        from concourse import bass2jax
        from concourse.bass2jax import _bass_exec_p, install_neuronx_cc_hook

        install_neuronx_cc_hook()
        self.jax = jax
        self.nc = nc
        self.n_cores = n_cores
        pname = nc.partition_id_tensor.name if nc.partition_id_tensor else None

        in_names, out_names, out_avals, zero_outs = [], [], [], []
        for alloc in nc.m.functions[0].allocations:
            if not isinstance(alloc, mybir.MemoryLocationSet):
                continue
            name = alloc.memorylocations[0].name
            if alloc.kind == "ExternalInput":
                if name != pname:
                    in_names.append(name)
            elif alloc.kind == "ExternalOutput":
                shape = tuple(alloc.tensor_shape)
                dtype = mybir.dt.np(alloc.dtype)
                out_names.append(name)
                out_avals.append(jax.core.ShapedArray(shape, dtype))
                zero_outs.append(np.zeros(shape, dtype))
        self.in_names, self.out_names = in_names, out_names
        self.out_avals, self.zero_outs = out_avals, zero_outs
        n_params = len(in_names)
        all_in = list(in_names) + list(out_names)
        if pname is not None:
            all_in.append(pname)

        def _body(*args):
            operands = list(args)
            if pname is not None:
                operands.append(bass2jax.partition_id_tensor())
            outs = _bass_exec_p.bind(
                *operands,
                out_avals=tuple(out_avals),
                in_names=tuple(all_in),
                out_names=tuple(out_names),
                lowering_input_output_aliases=(),
                sim_require_finite=False,
                sim_require_nnan=False,
                nc=nc,
            )
            return tuple(outs)

        devices = jax.devices()[:n_cores]
        self.mesh = Mesh(np.asarray(devices), ("core",))
        in_specs = (PartitionSpec("core"),) * (n_params + len(out_names))
        out_specs = (PartitionSpec("core"),) * len(out_names)
        self._fn = jax.jit(
            shard_map(
                _body,
                mesh=self.mesh,
                in_specs=in_specs,
                out_specs=out_specs,
                check_rep=False,
            ),
            keep_unused=True,
        )

    def put_inputs(self, in_maps):
        import jax
        from jax.sharding import PartitionSpec

        sh = jax.sharding.NamedSharding(self.mesh, PartitionSpec("core"))
        args = []
        for name in self.in_names:
            cat = np.concatenate([np.asarray(m[name]) for m in in_maps], axis=0)
            args.append(jax.device_put(cat, sh))
        for z in self.zero_outs:
            cat = np.zeros((self.n_cores * z.shape[0], *z.shape[1:]), z.dtype)
            args.append(jax.device_put(cat, sh))
        return args

    def run(self, args):
        outs = self._fn(*args)
        self.jax.block_until_ready(outs)
        return outs

    def results(self, outs):
        res = []
        for c in range(self.n_cores):
            d = {}
            for i, name in enumerate(self.out_names):
                full = np.asarray(outs[i])
                per = full.shape[0] // self.n_cores
                d[name] = full[c * per : (c + 1) * per]
            res.append(d)
        return res


_RUNNER_CACHE: dict = {}
LAST_RUN: dict = {}


# ----------------------------------------------------------------------------
# Host-side sharding / index prep
# ----------------------------------------------------------------------------
def _prepare(token_ids, segment_ids):
    tid = np.asarray(token_ids, dtype=np.int64)
    seg = np.asarray(segment_ids, dtype=np.int64)
    bounds = np.searchsorted(seg, np.arange(0, BATCH + 1, CHUNK_DOCS))

    groups = []  # [chunk][w] -> (ids int64 sorted, segrel int64)
    counts = np.zeros((N_CHUNKS, N_WIN), dtype=np.int64)
    for k in range(N_CHUNKS):
        s, e = bounds[k], bounds[k + 1]
        t_k = tid[s:e]
        g_k = seg[s:e] - k * CHUNK_DOCS
        order = np.argsort(t_k, kind="stable")
        t_k = t_k[order]
        g_k = g_k[order]
        wb = np.searchsorted(t_k, np.arange(0, (N_WIN + 1) * WIN, WIN))
        per_w = []
        for w in range(N_WIN):
            sl = slice(wb[w], wb[w + 1])
            per_w.append((t_k[sl] - w * WIN, g_k[sl]))
            counts[k, w] = wb[w + 1] - wb[w]
        groups.append(per_w)

    lw = tuple(
        int(np.ceil(counts[:, w].max() / 128.0) * 128) if counts[:, w].max() > 0
        else 128
        for w in range(N_WIN)
    )

    idx_cols = 4 * sum(lw) // 16
    seg_cols = 4 * sum(lw) // 128
    idx_all = np.zeros((N_CORES, 16, idx_cols), dtype=np.int16)
    seg_all = np.full((N_CORES, 128, seg_cols), -1.0, dtype=np.float32)
    for c in range(N_CORES):
        icol = 0
        scol = 0
        for ch in range(4):
            k = c * 4 + ch
            for w in range(N_WIN):
                ids_g, seg_g = groups[k][w]
                n = len(ids_g)
                l = lw[w]
                ids_p = np.zeros(l, dtype=np.int16)
                ids_p[:n] = ids_g.astype(np.int16)
                seg_p = np.full(l, -1.0, dtype=np.float32)
                seg_p[:n] = seg_g.astype(np.float32)
                idx_all[c, :, icol : icol + l // 16] = ids_p.reshape(-1, 16).T
                seg_all[c, :, scol : scol + l // 128] = seg_p.reshape(-1, 128).T
                icol += l // 16
                scol += l // 128
    idx_all = np.tile(idx_all, (1, 8, 1))  # replicate to all 8 Q7 core groups
    return lw, idx_all, seg_all


def kernel(token_ids, segment_ids, labels, emb, W1, b1, gamma, beta, W2, b2):
    lw, idx_all, seg_all = _prepare(token_ids, segment_ids)

    if lw not in _RUNNER_CACHE:
        nc = build_bass(lw)
        _RUNNER_CACHE[lw] = _Runner(nc, N_CORES)
    runner = _RUNNER_CACHE[lw]

    emb_f = np.ascontiguousarray(np.asarray(emb, dtype=np.float32))
    w1t = np.ascontiguousarray(np.asarray(W1, dtype=np.float32).T)
    b1c = np.asarray(b1, dtype=np.float32).reshape(HID, 1)
    gmc = np.asarray(gamma, dtype=np.float32).reshape(HID, 1)
    btc = np.asarray(beta, dtype=np.float32).reshape(HID, 1)
    w2c = np.asarray(W2, dtype=np.float32).reshape(1, HID).T.copy()
    b2c = np.asarray(b2, dtype=np.float32).reshape(1, 1)
    lab = np.asarray(labels, dtype=np.float32)
    iota = np.tile(np.arange(128, dtype=np.float32), (128, 1))

    in_maps = []
    for c in range(N_CORES):
        in_maps.append(
            {
                "emb": emb_f,
                "idx16": idx_all[c],
                "segf": seg_all[c],
                "w1t": w1t,
                "b1c": b1c,
                "gammac": gmc,
                "betac": btc,
                "w2c": w2c,
                "b2c": b2c,
                "labels": lab[c * DOCS_PER_CORE : (c + 1) * DOCS_PER_CORE].reshape(
                    1, -1
                ),
                "iota": iota,
            }
        )

    args = runner.put_inputs(in_maps)
    outs = runner.run(args)
    res = runner.results(outs)

    LAST_RUN["runner"] = runner
    LAST_RUN["args"] = args

    logits = np.concatenate([res[c]["logits"][0] for c in range(N_CORES)])
    loss = np.float32(res[0]["loss"][0, 0])
    return (np.asarray(loss, dtype=np.float32), logits.astype(np.float32))


# revision 10
# speedup vs baseline: 1.2723x; 1.2723x over previous
"""Trainium2 Bass kernel for nn_BOW_model (fused EmbeddingBag-mean -> Linear
-> BatchNorm1d -> ReLU -> Linear -> BCEWithLogitsLoss).

Sharding: data-parallel over documents. Each of the 8 cores owns 512
contiguous documents (segment ids are sorted). The embedding table and MLP
params are replicated. Per core: gather its tokens' embedding rows from HBM
(Ant dma_gather, int16 indices, so tokens are bucketed into four 32768-row
vocab windows and sorted within each window), segment-sum them into per-doc
sums + counts with a one-hot matmul on the tensor engine, then the tiny MLP.
BN batch stats (sum h, sum h^2) and the BCE loss sum are all-reduced across
cores on-device. Host only shards inputs / concatenates outputs.
"""
import numpy as np

import concourse.bacc as bacc
import concourse.mybir as mybir
import concourse.tile as tile
from concourse.masks import make_identity

VOCAB = 100000
HID = 256
BATCH = 4096
TOTAL_TOKENS = 819200
BN_EPS = 1e-5
N_CORES = 8
DOCS_PER_CORE = BATCH // N_CORES  # 512
CHUNK_DOCS = 128  # docs per PSUM accumulation chunk
N_CHUNKS = BATCH // CHUNK_DOCS  # 32 global, 4 per core
WIN = 32768  # vocab rows reachable by one int16 index window
N_WIN = 4  # ceil(100000 / 32768)
SUB = 8  # 128-row tiles per dma_gather call


# ----------------------------------------------------------------------------
# Workaround: this walrus build allows a single sync-wait command on the
# kernel-tail Drain, but Tile attaches one wait per in-flight proc. Absorb
# each pending wait into its own SP NOP ahead of a wait-free drain.
# ----------------------------------------------------------------------------
import bass_rust as _br


class _TileContextFixed(tile.TileContext):
    def _drain_and_barrier(self, tick_clock, wait_clock):
        gc = tick_clock.global_clock
        n = _br.N_PROCS
        for p in range(n):
            if gc[p] <= 0:
                continue
            partial = _br.VectorClock([gc[i] if i == p else 0 for i in range(n)])
            nop = self.nc.sync.nop(nofuse=True, hint=f"drain_absorb_p{p}")
            wait_clock.add_sem_waits(nop.ins, _br.ScopedClock({None: partial}))
        self.nc.sync.drain()
        self.nc.all_engine_barrier()
        assert self.sems is not None
        popped = self.nc._tile_sem_poison_stack.pop()
        assert popped is self._sem_poison
        self.nc.clear_and_free_semaphores(list(self.sems.allocated().values()))
        self.nc.all_engine_barrier()


# ----------------------------------------------------------------------------
# Device program
# ----------------------------------------------------------------------------
def build_bass(lw: tuple[int, ...], debug: bool = False, use_cc: bool = True, reps: int = 1):
    """lw[w] = padded token count per (chunk, vocab-window) group (mult of 128).

    Per-core inputs:
      emb    [VOCAB, HID] f32      (replicated)
      idx16  [128, 4*sum(lw)//16] int16  window-relative token ids, wrapped
             i -> [i%16, i//16] per group, groups ordered (chunk, w)
      segf   [128, 4*sum(lw)//128] f32   chunk-relative segment ids, wrapped
             i -> [i%128, i//128] per group (pad = -1)
      w1t    [HID, HID] f32  (= W1.T)
      b1c/gammac/betac/w2c [HID, 1] f32
      b2c    [1, 1] f32
      labels [1, DOCS_PER_CORE] f32
      iota   [128, 128] f32, iota[p, m] = m
    Outputs:
      logits [1, DOCS_PER_CORE] f32
      loss   [1, 1] f32 (already divided by BATCH; identical on all cores)
    """
    f32 = mybir.dt.float32
    group_tiles = [l // 128 for l in lw]
    tiles_per_chunk = sum(group_tiles)
    n_idx_cols = 4 * sum(lw) // 16
    n_seg_cols = 4 * tiles_per_chunk

    nc = bacc.Bacc("TRN2", target_bir_lowering=False, num_swdge_queues=4)
    emb = nc.declare_dram_parameter("emb", [VOCAB, HID], mybir.dt.float32r, isOutput=False)
    idx16 = nc.declare_dram_parameter(
        "idx16", [128, n_idx_cols], mybir.dt.int16, isOutput=False
    )
    segf = nc.declare_dram_parameter("segf", [128, n_seg_cols], f32, isOutput=False)
    w1t = nc.declare_dram_parameter("w1t", [HID, HID], f32, isOutput=False)
    b1c = nc.declare_dram_parameter("b1c", [HID, 1], f32, isOutput=False)
    gammac = nc.declare_dram_parameter("gammac", [HID, 1], f32, isOutput=False)
    betac = nc.declare_dram_parameter("betac", [HID, 1], f32, isOutput=False)
    w2c = nc.declare_dram_parameter("w2c", [HID, 1], f32, isOutput=False)
    b2c = nc.declare_dram_parameter("b2c", [1, 1], f32, isOutput=False)
    labels = nc.declare_dram_parameter(
        "labels", [1, DOCS_PER_CORE], f32, isOutput=False
    )
    iota_in = nc.declare_dram_parameter("iota", [128, 128], f32, isOutput=False)
    out_logits = nc.declare_dram_parameter(
        "logits", [1, DOCS_PER_CORE], f32, isOutput=True
    )
    out_loss = nc.declare_dram_parameter("loss", [1, 1], f32, isOutput=True)
    if debug:
        out_bow = nc.declare_dram_parameter("dbg_bow", [128, 4 * HID + 4], f32, isOutput=True)
        out_h = nc.declare_dram_parameter("dbg_h", [HID, DOCS_PER_CORE], f32, isOutput=True)

    AF = mybir.ActivationFunctionType
    ALU = mybir.AluOpType
    AX = mybir.AxisListType

    with _TileContextFixed(nc) as tc:
        with (
            tc.tile_pool(name="const", bufs=1) as cp,
            tc.tile_pool(name="gat", bufs=6) as gp,
            tc.tile_pool(name="oh", bufs=4) as ohp,
            tc.tile_pool(name="small", bufs=2) as sp,
            tc.tile_pool(name="psA", bufs=1, space="PSUM") as psA,
            tc.tile_pool(name="psT", bufs=2, space="PSUM") as psT,
            tc.tile_pool(name="psH", bufs=1, space="PSUM") as psH,
            tc.tile_pool(name="psL", bufs=1, space="PSUM") as psL,
            tc.tile_pool(name="dram", bufs=1, space="DRAM") as dp,
        ):
            # ---- constants / static tiles ----
            iota_sb = cp.tile([128, 128], f32, name="iota")
            nc.sync.dma_start(out=iota_sb[:], in_=iota_in[:])
            ones_sb = cp.tile([128, 1], f32, name="ones")
            nc.vector.memset(ones_sb[:], 1.0)
            ident = cp.tile([128, 128], f32, name="ident")
            make_identity(nc, ident[:])
            idx_sb = cp.tile([128, n_idx_cols], mybir.dt.int16, name="idx")
            nc.sync.dma_start(out=idx_sb[:], in_=idx16[:])
            seg_sb = cp.tile([128, n_seg_cols], f32, name="seg")
            nc.sync.dma_start(out=seg_sb[:], in_=segf[:])
            w1t_sb = [cp.tile([128, HID], f32, name=f"w1t{ih}") for ih in range(2)]
            for ih in range(2):
                nc.sync.dma_start(
                    out=w1t_sb[ih][:], in_=w1t[ih * 128 : (ih + 1) * 128, :]
                )
            b1_sb = [cp.tile([128, 1], f32, name=f"b1{j}") for j in range(2)]
            gm_sb = [cp.tile([128, 1], f32, name=f"gm{j}") for j in range(2)]
            bt_sb = [cp.tile([128, 1], f32, name=f"bt{j}") for j in range(2)]
            w2_sb = [cp.tile([128, 1], f32, name=f"w2{j}") for j in range(2)]
            for j in range(2):
                rows = slice(j * 128, (j + 1) * 128)
                nc.sync.dma_start(out=b1_sb[j][:], in_=b1c[rows, :])
                nc.sync.dma_start(out=gm_sb[j][:], in_=gammac[rows, :])
                nc.sync.dma_start(out=bt_sb[j][:], in_=betac[rows, :])
                nc.sync.dma_start(out=w2_sb[j][:], in_=w2c[rows, :])
            b2_sb = cp.tile([1, 1], f32, name="b2")
            nc.sync.dma_start(out=b2_sb[:], in_=b2c[:])
            lab_sb = cp.tile([1, DOCS_PER_CORE], f32, name="lab")
            nc.sync.dma_start(out=lab_sb[:], in_=labels[:])

            bow_sb = cp.tile([128, 4 * HID], f32, name="bow")
            eps_sb = cp.tile([128, 1], f32, name="eps")
            nc.vector.memset(eps_sb[:], BN_EPS)

            # ---- phase 1: gather + segment-sum per 128-doc chunk ----
            qn = 0
            for _rep in range(reps):
             for ch in range(4):
                ps = psA.tile([128, HID + 1], f32, name="acc")
                ohsum = sp.tile([128, 128], f32, name="ohsum")
                nc.vector.memset(ohsum[:], 0.0)
                tile_col0 = ch * tiles_per_chunk  # seg col base for this chunk
                idx_col0 = ch * sum(lw) // 16
                n_tiles_done = 0
                for w in range(4):
                    l = lw[w]
                    g_tiles = group_tiles[w]
                    # gather calls of up to SUB tiles each
                    emb_win = emb[w * WIN :, :]
                    done = 0
                    while done < g_tiles:
                        ct = min(SUB, g_tiles - done)
                        ni = ct * 128
                        g = gp.tile([128, SUB * HID], mybir.dt.float32r, name="g")
                        nc.gpsimd.dma_gather(
                            out_ap=g[:, : ct * HID].rearrange(
                                "p (s h) -> p s h", s=ct
                            ),
                            in_ap=emb_win,
                            idxs_ap=idx_sb[
                                :, idx_col0 + done * 8 : idx_col0 + (done + ct) * 8
                            ],
                            num_idxs=ni,
                            num_idxs_reg=ni,
                            elem_size=HID,
                            queue_num=qn,
                            single_packet=False,
                        )
                        qn = (qn + 1) % 4
                        for s in range(ct):
                            tcol = tile_col0 + n_tiles_done
                            oh = ohp.tile([128, 128], mybir.dt.float32r, name="oh")
                            nc.vector.tensor_scalar(
                                out=oh[:],
                                in0=iota_sb[:],
                                scalar1=seg_sb[:, tcol : tcol + 1],
                                scalar2=None,
                                op0=ALU.is_equal,
                            )
                            first = n_tiles_done == 0
                            last = n_tiles_done == tiles_per_chunk - 1
                            nc.tensor.matmul(
                                out=ps[:, :HID],
                                lhsT=oh[:],
                                rhs=g[:, s * HID : (s + 1) * HID],
                                start=first,
                                stop=last,
                            )
                            nc.vector.tensor_add(
                                out=ohsum[:], in0=ohsum[:], in1=oh[:]
                            )
                            n_tiles_done += 1
                        done += ct
                    idx_col0 += l // 16
                nc.tensor.matmul(
                    out=ps[:, HID : HID + 1],
                    lhsT=ohsum[:],
                    rhs=ones_sb[:],
                    start=True,
                    stop=True,
                )
                # bow = seg_sum / max(counts, 1)
                cnt = sp.tile([128, 1], f32, name="cnt")
                nc.vector.tensor_scalar_max(
                    out=cnt[:], in0=ps[:, HID : HID + 1], scalar1=1.0
                )
                if debug:
                    nc.sync.dma_start(
                        out=out_bow[:, 4 * HID + ch : 4 * HID + ch + 1], in_=cnt[:]
                    )
                rec = sp.tile([128, 1], f32, name="rec")
                nc.vector.reciprocal(out=rec[:], in_=cnt[:])
                nc.vector.tensor_scalar_mul(
                    out=bow_sb[:, ch * HID : (ch + 1) * HID],
                    in0=ps[:, :HID],
                    scalar1=rec[:],
                )

            if debug:
                nc.sync.dma_start(out=out_bow[:, : 4 * HID], in_=bow_sb[:])

            # ---- transpose bow [512 docs, 256] -> bowT (2 x [128, 512]) ----
            bowT = [cp.tile([128, 4 * 128], f32, name=f"bowT{j}") for j in range(2)]
            for ch in range(4):
                for fh in range(2):
                    pt = psT.tile([128, 128], f32, name="pt")
                    nc.tensor.transpose(
                        out=pt[:],
                        in_=bow_sb[:, ch * HID + fh * 128 : ch * HID + (fh + 1) * 128],
                        identity=ident[:],
                    )
                    nc.vector.tensor_copy(
                        out=bowT[fh][:, ch * 128 : (ch + 1) * 128], in_=pt[:]
                    )

            # ---- h^T = W1T @ bowT + b1; local BN stats ----
            h_sb = [cp.tile([128, DOCS_PER_CORE], f32, name=f"h{j}") for j in range(2)]
            stats = cp.tile([128, 4], f32, name="stats")
            sq = sp.tile([128, DOCS_PER_CORE], f32, name="sq")
            for jh in range(2):
                ph = psH.tile([128, DOCS_PER_CORE], f32, name="ph")
                for ih in range(2):
                    nc.tensor.matmul(
                        out=ph[:],
                        lhsT=w1t_sb[ih][:, jh * 128 : (jh + 1) * 128],
                        rhs=bowT[ih][:],
                        start=ih == 0,
                        stop=ih == 1,
                    )
                nc.vector.tensor_scalar_add(
                    out=h_sb[jh][:], in0=ph[:], scalar1=b1_sb[jh][:]
                )
                if debug:
                    nc.sync.dma_start(
                        out=out_h[jh * 128 : (jh + 1) * 128, :], in_=h_sb[jh][:]
                    )
                nc.vector.tensor_reduce(
                    out=stats[:, jh : jh + 1],
                    in_=h_sb[jh][:],
                    axis=AX.X,
                    op=ALU.add,
                )
                nc.vector.tensor_mul(out=sq[:], in0=h_sb[jh][:], in1=h_sb[jh][:])
                nc.vector.tensor_reduce(
                    out=stats[:, 2 + jh : 3 + jh], in_=sq[:], axis=AX.X, op=ALU.add
                )

            # ---- all-reduce BN stats across the 8 cores ----
            st_in = dp.tile([128, 4], f32, name="st_in")
            st_out = dp.tile([128, 4], f32, name="st_out")
            nc.sync.dma_start(out=st_in[:], in_=stats[:])
            if use_cc:
                nc.gpsimd.collective_compute(
                    "AllReduce",
                    ALU.add,
                    replica_groups=[list(range(N_CORES))],
                    ins=[st_in.opt()],
                    outs=[st_out.opt()],
                )
            else:
                nc.sync.dma_start(out=st_out[:], in_=st_in[:])
            stg = cp.tile([128, 4], f32, name="stg")
            nc.sync.dma_start(out=stg[:], in_=st_out[:])

            # ---- BN scale/shift; hn = relu(h*scale + shift); logits ----
            pl = psL.tile([1, DOCS_PER_CORE], f32, name="pl")
            hn = sp.tile([128, DOCS_PER_CORE], f32, name="hn")
            for jh in range(2):
                mu = sp.tile([128, 1], f32, name="mu")
                nc.vector.tensor_scalar_mul(
                    out=mu[:], in0=stg[:, jh : jh + 1], scalar1=1.0 / BATCH
                )
                ex2 = sp.tile([128, 1], f32, name="ex2")
                nc.vector.tensor_scalar_mul(
                    out=ex2[:], in0=stg[:, 2 + jh : 3 + jh], scalar1=1.0 / BATCH
                )
                var = sp.tile([128, 1], f32, name="var")
                nc.vector.tensor_mul(out=var[:], in0=mu[:], in1=mu[:])
                nc.vector.tensor_sub(out=var[:], in0=ex2[:], in1=var[:])
                sd = sp.tile([128, 1], f32, name="sd")
                # sd = sqrt(var + eps)
                nc.scalar.activation(out=sd[:], in_=var[:], func=AF.Sqrt, bias=eps_sb[:])
                rsd = sp.tile([128, 1], f32, name="rsd")
                nc.vector.reciprocal(out=rsd[:], in_=sd[:])
                scale = sp.tile([128, 1], f32, name="scale")
                nc.vector.tensor_mul(out=scale[:], in0=gm_sb[jh][:], in1=rsd[:])
                shift = sp.tile([128, 1], f32, name="shift")
                nc.vector.tensor_mul(out=shift[:], in0=mu[:], in1=scale[:])
                nc.vector.tensor_sub(out=shift[:], in0=bt_sb[jh][:], in1=shift[:])
                nc.scalar.activation(
                    out=hn[:],
                    in_=h_sb[jh][:],
                    func=AF.Relu,
                    bias=shift[:],
                    scale=scale[:],
                )
                nc.tensor.matmul(
                    out=pl[:],
                    lhsT=w2_sb[jh][:],
                    rhs=hn[:],
                    start=jh == 0,
                    stop=jh == 1,
                )

            logit = sp.tile([1, DOCS_PER_CORE], f32, name="logit")
            nc.vector.tensor_scalar_add(out=logit[:], in0=pl[:], scalar1=b2_sb[:])
            nc.sync.dma_start(out=out_logits[:], in_=logit[:])

            # ---- BCE loss: max(l,0) - l*y + softplus(-|l|) ----
            t_relu = sp.tile([1, DOCS_PER_CORE], f32, name="t_relu")
            nc.scalar.activation(out=t_relu[:], in_=logit[:], func=AF.Relu)
            t_ly = sp.tile([1, DOCS_PER_CORE], f32, name="t_ly")
            nc.vector.tensor_mul(out=t_ly[:], in0=logit[:], in1=lab_sb[:])
            t_abs = sp.tile([1, DOCS_PER_CORE], f32, name="t_abs")
            nc.scalar.activation(out=t_abs[:], in_=logit[:], func=AF.Abs)
            t_e = sp.tile([1, DOCS_PER_CORE], f32, name="t_e")
            nc.scalar.activation(out=t_e[:], in_=t_abs[:], func=AF.Exp, scale=-1.0)
            t_sp = sp.tile([1, DOCS_PER_CORE], f32, name="t_sp")
            nc.scalar.activation(out=t_sp[:], in_=t_e[:], func=AF.Ln, bias=1.0)
            nc.vector.tensor_sub(out=t_relu[:], in0=t_relu[:], in1=t_ly[:])
            nc.vector.tensor_add(out=t_relu[:], in0=t_relu[:], in1=t_sp[:])
            lsum = sp.tile([1, 1], f32, name="lsum")
            nc.vector.tensor_reduce(
                out=lsum[:], in_=t_relu[:], axis=AX.X, op=ALU.add
            )
            ls_in = dp.tile([1, 1], f32, name="ls_in")
            ls_out = dp.tile([1, 1], f32, name="ls_out")
            nc.sync.dma_start(out=ls_in[:], in_=lsum[:])
            if use_cc:
                nc.gpsimd.collective_compute(
                    "AllReduce",
                    ALU.add,
                    replica_groups=[list(range(N_CORES))],
                    ins=[ls_in.opt()],
                    outs=[ls_out.opt()],
                )
            else:
                nc.sync.dma_start(out=ls_out[:], in_=ls_in[:])
            lsg = sp.tile([1, 1], f32, name="lsg")
            nc.sync.dma_start(out=lsg[:], in_=ls_out[:])
            nc.vector.tensor_scalar_mul(
                out=lsg[:], in0=lsg[:], scalar1=1.0 / BATCH
            )
            nc.sync.dma_start(out=out_loss[:], in_=lsg[:])

    nc.compile()
    return nc


# ----------------------------------------------------------------------------
# PJRT runner (kept warm across kernel() calls)
# ----------------------------------------------------------------------------
class _Runner:
    def __init__(self, nc, n_cores):
        import jax
        from jax.sharding import Mesh, PartitionSpec
        from jax.experimental.shard_map import shard_map
        from concourse import bass2jax
        from concourse.bass2jax import _bass_exec_p, install_neuronx_cc_hook

        install_neuronx_cc_hook()
        self.jax = jax
        self.nc = nc
        self.n_cores = n_cores
        pname = nc.partition_id_tensor.name if nc.partition_id_tensor else None

        in_names, out_names, out_avals, zero_outs = [], [], [], []
        for alloc in nc.m.functions[0].allocations:
            if not isinstance(alloc, mybir.MemoryLocationSet):
                continue
            name = alloc.memorylocations[0].name
            if alloc.kind == "ExternalInput":
                if name != pname:
                    in_names.append(name)
            elif alloc.kind == "ExternalOutput":
                shape = tuple(alloc.tensor_shape)
                dtype = mybir.dt.np(alloc.dtype)
                out_names.append(name)
                out_avals.append(jax.core.ShapedArray(shape, dtype))
                zero_outs.append(np.zeros(shape, dtype))
        self.in_names, self.out_names = in_names, out_names
        self.out_avals, self.zero_outs = out_avals, zero_outs
        n_params = len(in_names)
        all_in = list(in_names) + list(out_names)
        if pname is not None:
            all_in.append(pname)

        def _body(*args):
            operands = list(args)
            if pname is not None:
                operands.append(bass2jax.partition_id_tensor())
            outs = _bass_exec_p.bind(
                *operands,
                out_avals=tuple(out_avals),
                in_names=tuple(all_in),
                out_names=tuple(out_names),
                lowering_input_output_aliases=(),
                sim_require_finite=False,
                sim_require_nnan=False,
                nc=nc,
            )
            return tuple(outs)

        devices = jax.devices()[:n_cores]
        self.mesh = Mesh(np.asarray(devices), ("core",))
        in_specs = (PartitionSpec("core"),) * (n_params + len(out_names))
        out_specs = (PartitionSpec("core"),) * len(out_names)
        self._fn = jax.jit(
            shard_map(
                _body,
                mesh=self.mesh,
                in_specs=in_specs,
                out_specs=out_specs,
                check_rep=False,
            ),
            keep_unused=True,
        )

    def put_inputs(self, in_maps):
        import jax
        from jax.sharding import PartitionSpec

        sh = jax.sharding.NamedSharding(self.mesh, PartitionSpec("core"))
        args = []
        for name in self.in_names:
            cat = np.concatenate([np.asarray(m[name]) for m in in_maps], axis=0)
            args.append(jax.device_put(cat, sh))
        for z in self.zero_outs:
            cat = np.zeros((self.n_cores * z.shape[0], *z.shape[1:]), z.dtype)
            args.append(jax.device_put(cat, sh))
        return args

    def run(self, args):
        outs = self._fn(*args)
        self.jax.block_until_ready(outs)
        return outs

    def results(self, outs):
        res = []
        for c in range(self.n_cores):
            d = {}
            for i, name in enumerate(self.out_names):
                full = np.asarray(outs[i])
                per = full.shape[0] // self.n_cores
                d[name] = full[c * per : (c + 1) * per]
            res.append(d)
        return res


_RUNNER_CACHE: dict = {}
LAST_RUN: dict = {}


# ----------------------------------------------------------------------------
# Host-side sharding / index prep
# ----------------------------------------------------------------------------
def _prepare(token_ids, segment_ids):
    tid = np.asarray(token_ids, dtype=np.int64)
    seg = np.asarray(segment_ids, dtype=np.int64)
    bounds = np.searchsorted(seg, np.arange(0, BATCH + 1, CHUNK_DOCS))

    groups = []  # [chunk][w] -> (ids int64 sorted, segrel int64)
    counts = np.zeros((N_CHUNKS, N_WIN), dtype=np.int64)
    for k in range(N_CHUNKS):
        s, e = bounds[k], bounds[k + 1]
        t_k = tid[s:e]
        g_k = seg[s:e] - k * CHUNK_DOCS
        order = np.argsort(t_k, kind="stable")
        t_k = t_k[order]
        g_k = g_k[order]
        wb = np.searchsorted(t_k, np.arange(0, (N_WIN + 1) * WIN, WIN))
        per_w = []
        for w in range(N_WIN):
            sl = slice(wb[w], wb[w + 1])
            per_w.append((t_k[sl] - w * WIN, g_k[sl]))
            counts[k, w] = wb[w + 1] - wb[w]
        groups.append(per_w)

    lw = tuple(
        int(np.ceil(counts[:, w].max() / 128.0) * 128) if counts[:, w].max() > 0
        else 128
        for w in range(N_WIN)
    )

    idx_cols = 4 * sum(lw) // 16
    seg_cols = 4 * sum(lw) // 128
    idx_all = np.zeros((N_CORES, 16, idx_cols), dtype=np.int16)
    seg_all = np.full((N_CORES, 128, seg_cols), -1.0, dtype=np.float32)
    for c in range(N_CORES):
        icol = 0
        scol = 0
        for ch in range(4):
            k = c * 4 + ch
            for w in range(N_WIN):
                ids_g, seg_g = groups[k][w]
                n = len(ids_g)
                l = lw[w]
                ids_p = np.zeros(l, dtype=np.int16)
                ids_p[:n] = ids_g.astype(np.int16)
                seg_p = np.full(l, -1.0, dtype=np.float32)
                seg_p[:n] = seg_g.astype(np.float32)
                idx_all[c, :, icol : icol + l // 16] = ids_p.reshape(-1, 16).T
                seg_all[c, :, scol : scol + l // 128] = seg_p.reshape(-1, 128).T
                icol += l // 16
                scol += l // 128
    idx_all = np.tile(idx_all, (1, 8, 1))  # replicate to all 8 Q7 core groups
    return lw, idx_all, seg_all


def kernel(token_ids, segment_ids, labels, emb, W1, b1, gamma, beta, W2, b2):
    lw, idx_all, seg_all = _prepare(token_ids, segment_ids)

    if lw not in _RUNNER_CACHE:
        nc = build_bass(lw)
        _RUNNER_CACHE[lw] = _Runner(nc, N_CORES)
    runner = _RUNNER_CACHE[lw]

    emb_f = np.ascontiguousarray(np.asarray(emb, dtype=np.float32))
    w1t = np.ascontiguousarray(np.asarray(W1, dtype=np.float32).T)
    b1c = np.asarray(b1, dtype=np.float32).reshape(HID, 1)
    gmc = np.asarray(gamma, dtype=np.float32).reshape(HID, 1)
    btc = np.asarray(beta, dtype=np.float32).reshape(HID, 1)
    w2c = np.asarray(W2, dtype=np.float32).reshape(1, HID).T.copy()
    b2c = np.asarray(b2, dtype=np.float32).reshape(1, 1)
    lab = np.asarray(labels, dtype=np.float32)
    iota = np.tile(np.arange(128, dtype=np.float32), (128, 1))

    in_maps = []
    for c in range(N_CORES):
        in_maps.append(
            {
                "emb": emb_f,
                "idx16": idx_all[c],
                "segf": seg_all[c],
                "w1t": w1t,
                "b1c": b1c,
                "gammac": gmc,
                "betac": btc,
                "w2c": w2c,
                "b2c": b2c,
                "labels": lab[c * DOCS_PER_CORE : (c + 1) * DOCS_PER_CORE].reshape(
                    1, -1
                ),
                "iota": iota,
            }
        )

    args = runner.put_inputs(in_maps)
    outs = runner.run(args)
    res = runner.results(outs)

    LAST_RUN["runner"] = runner
    LAST_RUN["args"] = args

    logits = np.concatenate([res[c]["logits"][0] for c in range(N_CORES)])
    loss = np.float32(res[0]["loss"][0, 0])
    return (np.asarray(loss, dtype=np.float32), logits.astype(np.float32))


# revision 11
# speedup vs baseline: 1.6429x; 1.2913x over previous
"""Trainium2 Bass kernel for nn_BOW_model (fused EmbeddingBag-mean -> Linear
-> BatchNorm1d -> ReLU -> Linear -> BCEWithLogitsLoss).

Sharding: data-parallel over documents. Each of the 8 cores owns 512
contiguous documents (segment ids are sorted). The embedding table and MLP
params are replicated. Per core: gather its tokens' embedding rows from HBM
(Ant dma_gather, int16 indices, so tokens are bucketed into four 32768-row
vocab windows and sorted within each window), segment-sum them into per-doc
sums + counts with a one-hot matmul on the tensor engine, then the tiny MLP.
BN batch stats (sum h, sum h^2) and the BCE loss sum are all-reduced across
cores on-device. Host only shards inputs / concatenates outputs.
"""
import numpy as np

import concourse.bacc as bacc
import concourse.mybir as mybir
import concourse.tile as tile
from concourse.masks import make_identity

VOCAB = 100000
HID = 256
BATCH = 4096
TOTAL_TOKENS = 819200
BN_EPS = 1e-5
N_CORES = 8
DOCS_PER_CORE = BATCH // N_CORES  # 512
CHUNK_DOCS = 128  # docs per PSUM accumulation chunk
N_CHUNKS = BATCH // CHUNK_DOCS  # 32 global, 4 per core
WIN = 32768  # vocab rows reachable by one int16 index window
N_WIN = 4  # ceil(100000 / 32768)
SUB = 8  # 128-row tiles per dma_gather call


# ----------------------------------------------------------------------------
# Workaround: this walrus build allows a single sync-wait command on the
# kernel-tail Drain, but Tile attaches one wait per in-flight proc. Absorb
# each pending wait into its own SP NOP ahead of a wait-free drain.
# ----------------------------------------------------------------------------
import bass_rust as _br


class _TileContextFixed(tile.TileContext):
    def _drain_and_barrier(self, tick_clock, wait_clock):
        gc = tick_clock.global_clock
        n = _br.N_PROCS
        for p in range(n):
            if gc[p] <= 0:
                continue
            partial = _br.VectorClock([gc[i] if i == p else 0 for i in range(n)])
            nop = self.nc.sync.nop(nofuse=True, hint=f"drain_absorb_p{p}")
            wait_clock.add_sem_waits(nop.ins, _br.ScopedClock({None: partial}))
        self.nc.sync.drain()
        self.nc.all_engine_barrier()
        assert self.sems is not None
        popped = self.nc._tile_sem_poison_stack.pop()
        assert popped is self._sem_poison
        self.nc.clear_and_free_semaphores(list(self.sems.allocated().values()))
        self.nc.all_engine_barrier()


# ----------------------------------------------------------------------------
# Device program
# ----------------------------------------------------------------------------
def build_bass(lw: tuple[int, ...], debug: bool = False, use_cc: bool = True, reps: int = 1):
    """lw[w] = padded token count per (chunk, vocab-window) group (mult of 128).

    Per-core inputs:
      emb    [VOCAB, HID] f32      (replicated)
      idx16  [128, 4*sum(lw)//16] int16  window-relative token ids, wrapped
             i -> [i%16, i//16] per group, groups ordered (chunk, w)
      segf   [128, 4*sum(lw)//128] f32   chunk-relative segment ids, wrapped
             i -> [i%128, i//128] per group (pad = -1)
      w1t    [HID, HID] f32  (= W1.T)
      b1c/gammac/betac/w2c [HID, 1] f32
      b2c    [1, 1] f32
      labels [1, DOCS_PER_CORE] f32
      iota   [128, 128] f32, iota[p, m] = m
    Outputs:
      logits [1, DOCS_PER_CORE] f32
      loss   [1, 1] f32 (already divided by BATCH; identical on all cores)
    """
    f32 = mybir.dt.float32
    group_tiles = [l // 128 for l in lw]
    tiles_per_chunk = sum(group_tiles)
    n_idx_cols = 4 * sum(lw) // 16
    n_seg_cols = 4 * tiles_per_chunk

    nc = bacc.Bacc("TRN2", target_bir_lowering=False, num_swdge_queues=4)
    emb = nc.declare_dram_parameter("emb", [VOCAB, HID], mybir.dt.float32r, isOutput=False)
    idx16 = nc.declare_dram_parameter(
        "idx16", [128, n_idx_cols], mybir.dt.int16, isOutput=False
    )
    segf = nc.declare_dram_parameter("segf", [128, n_seg_cols], f32, isOutput=False)
    w1t = nc.declare_dram_parameter("w1t", [HID, HID], f32, isOutput=False)
    b1c = nc.declare_dram_parameter("b1c", [HID, 1], f32, isOutput=False)
    gammac = nc.declare_dram_parameter("gammac", [HID, 1], f32, isOutput=False)
    betac = nc.declare_dram_parameter("betac", [HID, 1], f32, isOutput=False)
    w2c = nc.declare_dram_parameter("w2c", [HID, 1], f32, isOutput=False)
    b2c = nc.declare_dram_parameter("b2c", [1, 1], f32, isOutput=False)
    labels = nc.declare_dram_parameter(
        "labels", [1, DOCS_PER_CORE], f32, isOutput=False
    )
    iota_in = nc.declare_dram_parameter("iota", [128, 128], f32, isOutput=False)
    out_logits = nc.declare_dram_parameter(
        "logits", [1, DOCS_PER_CORE], f32, isOutput=True
    )
    out_loss = nc.declare_dram_parameter("loss", [1, 1], f32, isOutput=True)
    if debug:
        out_bow = nc.declare_dram_parameter("dbg_bow", [128, 4 * HID + 4], f32, isOutput=True)
        out_h = nc.declare_dram_parameter("dbg_h", [HID, DOCS_PER_CORE], f32, isOutput=True)

    AF = mybir.ActivationFunctionType
    ALU = mybir.AluOpType
    AX = mybir.AxisListType

    with _TileContextFixed(nc) as tc:
        with (
            tc.tile_pool(name="const", bufs=1) as cp,
            tc.tile_pool(name="gat", bufs=6) as gp,
            tc.tile_pool(name="oh", bufs=4) as ohp,
            tc.tile_pool(name="small", bufs=2) as sp,
            tc.tile_pool(name="psA", bufs=1, space="PSUM") as psA,
            tc.tile_pool(name="psT", bufs=2, space="PSUM") as psT,
            tc.tile_pool(name="psH", bufs=1, space="PSUM") as psH,
            tc.tile_pool(name="psL", bufs=1, space="PSUM") as psL,
            tc.tile_pool(name="dram", bufs=1, space="DRAM") as dp,
        ):
            # ---- constants / static tiles ----
            iota_sb = cp.tile([128, 128], f32, name="iota")
            nc.sync.dma_start(out=iota_sb[:], in_=iota_in[:])
            ones_sb = cp.tile([128, 1], f32, name="ones")
            nc.vector.memset(ones_sb[:], 1.0)
            ident = cp.tile([128, 128], f32, name="ident")
            make_identity(nc, ident[:])
            idx_sb = cp.tile([128, n_idx_cols], mybir.dt.int16, name="idx")
            nc.sync.dma_start(out=idx_sb[:], in_=idx16[:])
            seg_sb = cp.tile([128, n_seg_cols], f32, name="seg")
            nc.sync.dma_start(out=seg_sb[:], in_=segf[:])
            w1t_sb = [cp.tile([128, HID], f32, name=f"w1t{ih}") for ih in range(2)]
            for ih in range(2):
                nc.sync.dma_start(
                    out=w1t_sb[ih][:], in_=w1t[ih * 128 : (ih + 1) * 128, :]
                )
            b1_sb = [cp.tile([128, 1], f32, name=f"b1{j}") for j in range(2)]
            gm_sb = [cp.tile([128, 1], f32, name=f"gm{j}") for j in range(2)]
            bt_sb = [cp.tile([128, 1], f32, name=f"bt{j}") for j in range(2)]
            w2_sb = [cp.tile([128, 1], f32, name=f"w2{j}") for j in range(2)]
            for j in range(2):
                rows = slice(j * 128, (j + 1) * 128)
                nc.sync.dma_start(out=b1_sb[j][:], in_=b1c[rows, :])
                nc.sync.dma_start(out=gm_sb[j][:], in_=gammac[rows, :])
                nc.sync.dma_start(out=bt_sb[j][:], in_=betac[rows, :])
                nc.sync.dma_start(out=w2_sb[j][:], in_=w2c[rows, :])
            b2_sb = cp.tile([1, 1], f32, name="b2")
            nc.sync.dma_start(out=b2_sb[:], in_=b2c[:])
            lab_sb = cp.tile([1, DOCS_PER_CORE], f32, name="lab")
            nc.sync.dma_start(out=lab_sb[:], in_=labels[:])

            bow_sb = cp.tile([128, 4 * HID], f32, name="bow")
            eps_sb = cp.tile([128, 1], f32, name="eps")
            nc.vector.memset(eps_sb[:], BN_EPS)

            # ---- phase 1: gather + segment-sum per 128-doc chunk ----
            qn = 0
            for _rep in range(reps):
             for ch in range(4):
                ps = psA.tile([128, HID + 1], f32, name="acc")
                ohsum_w = sp.tile([128, SUB * 128], f32, name="ohsum_w")
                nc.vector.memset(ohsum_w[:], 0.0)
                tile_col0 = ch * tiles_per_chunk  # seg col base for this chunk
                idx_col0 = ch * sum(lw) // 16
                n_tiles_done = 0
                for w in range(4):
                    l = lw[w]
                    g_tiles = group_tiles[w]
                    # gather calls of up to SUB tiles each
                    emb_win = emb[w * WIN :, :]
                    done = 0
                    while done < g_tiles:
                        ct = min(SUB, g_tiles - done)
                        ni = ct * 128
                        g = gp.tile([128, SUB * HID], mybir.dt.float32r, name="g")
                        nc.gpsimd.dma_gather(
                            out_ap=g[:, : ct * HID].rearrange(
                                "p (s h) -> p s h", s=ct
                            ),
                            in_ap=emb_win,
                            idxs_ap=idx_sb[
                                :, idx_col0 + done * 8 : idx_col0 + (done + ct) * 8
                            ],
                            num_idxs=ni,
                            num_idxs_reg=ni,
                            elem_size=HID,
                            queue_num=qn,
                            single_packet=False,
                        )
                        qn = (qn + 1) % 4
                        tcol = tile_col0 + n_tiles_done
                        ohb = ohp.tile(
                            [128, SUB * 128], mybir.dt.float32r, name="ohb"
                        )
                        nc.vector.tensor_tensor(
                            out=ohb[:, : ct * 128].rearrange(
                                "p (s m) -> p s m", s=ct
                            ),
                            in0=iota_sb[:]
                            .rearrange("p (o m) -> p o m", o=1)
                            .broadcast_to([128, ct, 128]),
                            in1=seg_sb[:, tcol : tcol + ct]
                            .rearrange("p (s o) -> p s o", o=1)
                            .broadcast_to([128, ct, 128]),
                            op=ALU.is_equal,
                        )
                        for s in range(ct):
                            first = n_tiles_done == 0
                            last = n_tiles_done == tiles_per_chunk - 1
                            nc.tensor.matmul(
                                out=ps[:, :HID],
                                lhsT=ohb[:, s * 128 : (s + 1) * 128],
                                rhs=g[:, s * HID : (s + 1) * HID],
                                start=first,
                                stop=last,
                            )
                            n_tiles_done += 1
                        nc.vector.tensor_add(
                            out=ohsum_w[:, : ct * 128],
                            in0=ohsum_w[:, : ct * 128],
                            in1=ohb[:, : ct * 128],
                        )
                        done += ct
                    idx_col0 += l // 16
                for s in range(1, SUB):
                    nc.vector.tensor_add(
                        out=ohsum_w[:, :128],
                        in0=ohsum_w[:, :128],
                        in1=ohsum_w[:, s * 128 : (s + 1) * 128],
                    )
                nc.tensor.matmul(
                    out=ps[:, HID : HID + 1],
                    lhsT=ohsum_w[:, :128],
                    rhs=ones_sb[:],
                    start=True,
                    stop=True,
                )
                # bow = seg_sum / max(counts, 1)
                cnt = sp.tile([128, 1], f32, name="cnt")
                nc.vector.tensor_scalar_max(
                    out=cnt[:], in0=ps[:, HID : HID + 1], scalar1=1.0
                )
                if debug:
                    nc.sync.dma_start(
                        out=out_bow[:, 4 * HID + ch : 4 * HID + ch + 1], in_=cnt[:]
                    )
                rec = sp.tile([128, 1], f32, name="rec")
                nc.vector.reciprocal(out=rec[:], in_=cnt[:])
                nc.vector.tensor_scalar_mul(
                    out=bow_sb[:, ch * HID : (ch + 1) * HID],
                    in0=ps[:, :HID],
                    scalar1=rec[:],
                )

            if debug:
                nc.sync.dma_start(out=out_bow[:, : 4 * HID], in_=bow_sb[:])

            # ---- transpose bow [512 docs, 256] -> bowT (2 x [128, 512]) ----
            bowT = [cp.tile([128, 4 * 128], f32, name=f"bowT{j}") for j in range(2)]
            for ch in range(4):
                for fh in range(2):
                    pt = psT.tile([128, 128], f32, name="pt")
                    nc.tensor.transpose(
                        out=pt[:],
                        in_=bow_sb[:, ch * HID + fh * 128 : ch * HID + (fh + 1) * 128],
                        identity=ident[:],
                    )
                    nc.vector.tensor_copy(
                        out=bowT[fh][:, ch * 128 : (ch + 1) * 128], in_=pt[:]
                    )

            # ---- h^T = W1T @ bowT + b1; local BN stats ----
            h_sb = [cp.tile([128, DOCS_PER_CORE], f32, name=f"h{j}") for j in range(2)]
            stats = cp.tile([128, 4], f32, name="stats")
            sq = sp.tile([128, DOCS_PER_CORE], f32, name="sq")
            for jh in range(2):
                ph = psH.tile([128, DOCS_PER_CORE], f32, name="ph")
                for ih in range(2):
                    nc.tensor.matmul(
                        out=ph[:],
                        lhsT=w1t_sb[ih][:, jh * 128 : (jh + 1) * 128],
                        rhs=bowT[ih][:],
                        start=ih == 0,
                        stop=ih == 1,
                    )
                nc.vector.tensor_scalar_add(
                    out=h_sb[jh][:], in0=ph[:], scalar1=b1_sb[jh][:]
                )
                if debug:
                    nc.sync.dma_start(
                        out=out_h[jh * 128 : (jh + 1) * 128, :], in_=h_sb[jh][:]
                    )
                nc.vector.tensor_reduce(
                    out=stats[:, jh : jh + 1],
                    in_=h_sb[jh][:],
                    axis=AX.X,
                    op=ALU.add,
                )
                nc.vector.tensor_mul(out=sq[:], in0=h_sb[jh][:], in1=h_sb[jh][:])
                nc.vector.tensor_reduce(
                    out=stats[:, 2 + jh : 3 + jh], in_=sq[:], axis=AX.X, op=ALU.add
                )

            # ---- all-reduce BN stats across the 8 cores ----
            st_in = dp.tile([128, 4], f32, name="st_in")
            st_out = dp.tile([128, 4], f32, name="st_out")
            nc.sync.dma_start(out=st_in[:], in_=stats[:])
            if use_cc:
                nc.gpsimd.collective_compute(
                    "AllReduce",
                    ALU.add,
                    replica_groups=[list(range(N_CORES))],
                    ins=[st_in.opt()],
                    outs=[st_out.opt()],
                )
            else:
                nc.sync.dma_start(out=st_out[:], in_=st_in[:])
            stg = cp.tile([128, 4], f32, name="stg")
            nc.sync.dma_start(out=stg[:], in_=st_out[:])

            # ---- BN scale/shift; hn = relu(h*scale + shift); logits ----
            pl = psL.tile([1, DOCS_PER_CORE], f32, name="pl")
            hn = sp.tile([128, DOCS_PER_CORE], f32, name="hn")
            for jh in range(2):
                mu = sp.tile([128, 1], f32, name="mu")
                nc.vector.tensor_scalar_mul(
                    out=mu[:], in0=stg[:, jh : jh + 1], scalar1=1.0 / BATCH
                )
                ex2 = sp.tile([128, 1], f32, name="ex2")
                nc.vector.tensor_scalar_mul(
                    out=ex2[:], in0=stg[:, 2 + jh : 3 + jh], scalar1=1.0 / BATCH
                )
                var = sp.tile([128, 1], f32, name="var")
                nc.vector.tensor_mul(out=var[:], in0=mu[:], in1=mu[:])
                nc.vector.tensor_sub(out=var[:], in0=ex2[:], in1=var[:])
                sd = sp.tile([128, 1], f32, name="sd")
                # sd = sqrt(var + eps)
                nc.scalar.activation(out=sd[:], in_=var[:], func=AF.Sqrt, bias=eps_sb[:])
                rsd = sp.tile([128, 1], f32, name="rsd")
                nc.vector.reciprocal(out=rsd[:], in_=sd[:])
                scale = sp.tile([128, 1], f32, name="scale")
                nc.vector.tensor_mul(out=scale[:], in0=gm_sb[jh][:], in1=rsd[:])
                shift = sp.tile([128, 1], f32, name="shift")
                nc.vector.tensor_mul(out=shift[:], in0=mu[:], in1=scale[:])
                nc.vector.tensor_sub(out=shift[:], in0=bt_sb[jh][:], in1=shift[:])
                nc.scalar.activation(
                    out=hn[:],
                    in_=h_sb[jh][:],
                    func=AF.Relu,
                    bias=shift[:],
                    scale=scale[:],
                )
                nc.tensor.matmul(
                    out=pl[:],
                    lhsT=w2_sb[jh][:],
                    rhs=hn[:],
                    start=jh == 0,
                    stop=jh == 1,
                )

            logit = sp.tile([1, DOCS_PER_CORE], f32, name="logit")
            nc.vector.tensor_scalar_add(out=logit[:], in0=pl[:], scalar1=b2_sb[:])
            nc.sync.dma_start(out=out_logits[:], in_=logit[:])

            # ---- BCE loss: max(l,0) - l*y + softplus(-|l|) ----
            t_relu = sp.tile([1, DOCS_PER_CORE], f32, name="t_relu")
            nc.scalar.activation(out=t_relu[:], in_=logit[:], func=AF.Relu)
            t_ly = sp.tile([1, DOCS_PER_CORE], f32, name="t_ly")
            nc.vector.tensor_mul(out=t_ly[:], in0=logit[:], in1=lab_sb[:])
            t_abs = sp.tile([1, DOCS_PER_CORE], f32, name="t_abs")
            nc.scalar.activation(out=t_abs[:], in_=logit[:], func=AF.Abs)
            t_e = sp.tile([1, DOCS_PER_CORE], f32, name="t_e")
            nc.scalar.activation(out=t_e[:], in_=t_abs[:], func=AF.Exp, scale=-1.0)
            t_sp = sp.tile([1, DOCS_PER_CORE], f32, name="t_sp")
            nc.scalar.activation(out=t_sp[:], in_=t_e[:], func=AF.Ln, bias=1.0)
            nc.vector.tensor_sub(out=t_relu[:], in0=t_relu[:], in1=t_ly[:])
            nc.vector.tensor_add(out=t_relu[:], in0=t_relu[:], in1=t_sp[:])
            lsum = sp.tile([1, 1], f32, name="lsum")
            nc.vector.tensor_reduce(
                out=lsum[:], in_=t_relu[:], axis=AX.X, op=ALU.add
            )
            ls_in = dp.tile([1, 1], f32, name="ls_in")
            ls_out = dp.tile([1, 1], f32, name="ls_out")
            nc.sync.dma_start(out=ls_in[:], in_=lsum[:])
            if use_cc:
                nc.gpsimd.collective_compute(
                    "AllReduce",
                    ALU.add,
                    replica_groups=[list(range(N_CORES))],
                    ins=[ls_in.opt()],
                    outs=[ls_out.opt()],
                )
            else:
                nc.sync.dma_start(out=ls_out[:], in_=ls_in[:])
            lsg = sp.tile([1, 1], f32, name="lsg")
            nc.sync.dma_start(out=lsg[:], in_=ls_out[:])
            nc.vector.tensor_scalar_mul(
                out=lsg[:], in0=lsg[:], scalar1=1.0 / BATCH
            )
            nc.sync.dma_start(out=out_loss[:], in_=lsg[:])

    nc.compile()
    return nc


# ----------------------------------------------------------------------------
# PJRT runner (kept warm across kernel() calls)
# ----------------------------------------------------------------------------
class _Runner:
    def __init__(self, nc, n_cores):
        import jax
        from jax.sharding import Mesh, PartitionSpec
        from jax.experimental.shard_map import shard_map
        from concourse import bass2jax
        from concourse.bass2jax import _bass_exec_p, install_neuronx_cc_hook

        install_neuronx_cc_hook()
        self.jax = jax
        self.nc = nc
        self.n_cores = n_cores
        pname = nc.partition_id_tensor.name if nc.partition_id_tensor else None

        in_names, out_names, out_avals, zero_outs = [], [], [], []
        for alloc in nc.m.functions[0].allocations:
            if not isinstance(alloc, mybir.MemoryLocationSet):
                continue
            name = alloc.memorylocations[0].name
            if alloc.kind == "ExternalInput":
                if name != pname:
                    in_names.append(name)
            elif alloc.kind == "ExternalOutput":
                shape = tuple(alloc.tensor_shape)
                dtype = mybir.dt.np(alloc.dtype)
                out_names.append(name)
                out_avals.append(jax.core.ShapedArray(shape, dtype))
                zero_outs.append(np.zeros(shape, dtype))
        self.in_names, self.out_names = in_names, out_names
        self.out_avals, self.zero_outs = out_avals, zero_outs
        n_params = len(in_names)
        all_in = list(in_names) + list(out_names)
        if pname is not None:
            all_in.append(pname)

        def _body(*args):
            operands = list(args)
            if pname is not None:
                operands.append(bass2jax.partition_id_tensor())
            outs = _bass_exec_p.bind(
                *operands,
                out_avals=tuple(out_avals),
                in_names=tuple(all_in),
                out_names=tuple(out_names),
                lowering_input_output_aliases=(),
                sim_require_finite=False,
                sim_require_nnan=False,
                nc=nc,
            )
            return tuple(outs)

        devices = jax.devices()[:n_cores]
        self.mesh = Mesh(np.asarray(devices), ("core",))
        in_specs = (PartitionSpec("core"),) * (n_params + len(out_names))
        out_specs = (PartitionSpec("core"),) * len(out_names)
        self._fn = jax.jit(
            shard_map(
                _body,
                mesh=self.mesh,
                in_specs=in_specs,
                out_specs=out_specs,
                check_rep=False,
            ),
            keep_unused=True,
        )

    def put_inputs(self, in_maps):
        import jax
        from jax.sharding import PartitionSpec

        sh = jax.sharding.NamedSharding(self.mesh, PartitionSpec("core"))
        args = []
        for name in self.in_names:
            cat = np.concatenate([np.asarray(m[name]) for m in in_maps], axis=0)
            args.append(jax.device_put(cat, sh))
        for z in self.zero_outs:
            cat = np.zeros((self.n_cores * z.shape[0], *z.shape[1:]), z.dtype)
            args.append(jax.device_put(cat, sh))
        return args

    def run(self, args):
        outs = self._fn(*args)
        self.jax.block_until_ready(outs)
        return outs

    def results(self, outs):
        res = []
        for c in range(self.n_cores):
            d = {}
            for i, name in enumerate(self.out_names):
                full = np.asarray(outs[i])
                per = full.shape[0] // self.n_cores
                d[name] = full[c * per : (c + 1) * per]
            res.append(d)
        return res


_RUNNER_CACHE: dict = {}
LAST_RUN: dict = {}


# ----------------------------------------------------------------------------
# Host-side sharding / index prep
# ----------------------------------------------------------------------------
def _prepare(token_ids, segment_ids):
    tid = np.asarray(token_ids, dtype=np.int64)
    seg = np.asarray(segment_ids, dtype=np.int64)
    bounds = np.searchsorted(seg, np.arange(0, BATCH + 1, CHUNK_DOCS))

    groups = []  # [chunk][w] -> (ids int64 sorted, segrel int64)
    counts = np.zeros((N_CHUNKS, N_WIN), dtype=np.int64)
    for k in range(N_CHUNKS):
        s, e = bounds[k], bounds[k + 1]
        t_k = tid[s:e]
        g_k = seg[s:e] - k * CHUNK_DOCS
        order = np.argsort(t_k, kind="stable")
        t_k = t_k[order]
        g_k = g_k[order]
        wb = np.searchsorted(t_k, np.arange(0, (N_WIN + 1) * WIN, WIN))
        per_w = []
        for w in range(N_WIN):
            sl = slice(wb[w], wb[w + 1])
            per_w.append((t_k[sl] - w * WIN, g_k[sl]))
            counts[k, w] = wb[w + 1] - wb[w]
        groups.append(per_w)

    lw = tuple(
        int(np.ceil(counts[:, w].max() / 128.0) * 128) if counts[:, w].max() > 0
        else 128
        for w in range(N_WIN)
    )

    idx_cols = 4 * sum(lw) // 16
    seg_cols = 4 * sum(lw) // 128
    idx_all = np.zeros((N_CORES, 16, idx_cols), dtype=np.int16)
    seg_all = np.full((N_CORES, 128, seg_cols), -1.0, dtype=np.float32)
    for c in range(N_CORES):
        icol = 0
        scol = 0
        for ch in range(4):
            k = c * 4 + ch
            for w in range(N_WIN):
                ids_g, seg_g = groups[k][w]
                n = len(ids_g)
                l = lw[w]
                ids_p = np.zeros(l, dtype=np.int16)
                ids_p[:n] = ids_g.astype(np.int16)
                seg_p = np.full(l, -1.0, dtype=np.float32)
                seg_p[:n] = seg_g.astype(np.float32)
                idx_all[c, :, icol : icol + l // 16] = ids_p.reshape(-1, 16).T
                seg_all[c, :, scol : scol + l // 128] = seg_p.reshape(-1, 128).T
                icol += l // 16
                scol += l // 128
    idx_all = np.tile(idx_all, (1, 8, 1))  # replicate to all 8 Q7 core groups
    return lw, idx_all, seg_all


def kernel(token_ids, segment_ids, labels, emb, W1, b1, gamma, beta, W2, b2):
    lw, idx_all, seg_all = _prepare(token_ids, segment_ids)

    if lw not in _RUNNER_CACHE:
        nc = build_bass(lw)
        _RUNNER_CACHE[lw] = _Runner(nc, N_CORES)
    runner = _RUNNER_CACHE[lw]

    emb_f = np.ascontiguousarray(np.asarray(emb, dtype=np.float32))
    w1t = np.ascontiguousarray(np.asarray(W1, dtype=np.float32).T)
    b1c = np.asarray(b1, dtype=np.float32).reshape(HID, 1)
    gmc = np.asarray(gamma, dtype=np.float32).reshape(HID, 1)
    btc = np.asarray(beta, dtype=np.float32).reshape(HID, 1)
    w2c = np.asarray(W2, dtype=np.float32).reshape(1, HID).T.copy()
    b2c = np.asarray(b2, dtype=np.float32).reshape(1, 1)
    lab = np.asarray(labels, dtype=np.float32)
    iota = np.tile(np.arange(128, dtype=np.float32), (128, 1))

    in_maps = []
    for c in range(N_CORES):
        in_maps.append(
            {
                "emb": emb_f,
                "idx16": idx_all[c],
                "segf": seg_all[c],
                "w1t": w1t,
                "b1c": b1c,
                "gammac": gmc,
                "betac": btc,
                "w2c": w2c,
                "b2c": b2c,
                "labels": lab[c * DOCS_PER_CORE : (c + 1) * DOCS_PER_CORE].reshape(
                    1, -1
                ),
                "iota": iota,
            }
        )

    args = runner.put_inputs(in_maps)
    outs = runner.run(args)
    res = runner.results(outs)

    LAST_RUN["runner"] = runner
    LAST_RUN["args"] = args

    logits = np.concatenate([res[c]["logits"][0] for c in range(N_CORES)])
    loss = np.float32(res[0]["loss"][0, 0])
    return (np.asarray(loss, dtype=np.float32), logits.astype(np.float32))
